# revision 25
# baseline (speedup 1.0000x reference)
"""AttMaxPool2D (2x2 softmax-attention pooling) Trainium2 Bass kernel.

Problem: x [16, 224, 224, 128] f32 NHWC -> out [16, 112, 112, 128]
  patches = 2x2 non-overlapping windows; out = sum(p * softmax(p, axis=window)).

Sharding: pure data parallel over batch: 8 cores x 2 examples each.

Layout: each SBUF partition owns a QUARTER of one output-row-pair
(224 row-pairs x 4 quarters = 896 units = 7 full blocks of 128 partitions, no
idle lanes).  Free dim = segments of the input row-pair quarter; even/odd
input row segments are packed [0:fl] / [fl:2fl] per partition.

The kernel is memory-bound (64.2 MB/core at ~360 GB/s ~= 178 us), so the
engine split keeps every compute engine under that roofline (tolerance gate
is 2e-2, so bf16 intermediates are fine):
  ACT: E = exp(x) (f32 -> bf16), then r = exp(-ln(s)) one chunk deferred
       (Ln/Exp share one table set; DVE iterative divide would be slower).
  DVE: only the products: mAB = x_even * E_even, mCD = x_odd * E_odd
       (f32 x bf16 -> bf16, 1x mode), and the final out = n * r.
  PE:  all window sums via identity-stationary matmuls accumulating into
       PSUM (fp32 accumulate, exact):  n = m1+m2+m3+m4, s = EA+EB+EC+ED.
       Group width 512 f32 = one PSUM bank; n and s each use 4 banks.
  Store: SWDGE (gpsimd) DMA with bf16->f32 cast; loads issued from SP (sync)
       so no DMA-issue time lands on ACT/DVE.
"""

import os
from contextlib import ExitStack

import numpy as np

import concourse.bass as bass
import concourse.mybir as mybir
import concourse.tile as tile
from concourse.masks import make_identity

F32 = mybir.dt.float32
BF16 = mybir.dt.bfloat16

# Full problem shape (hardcoded per contract).
B, H, W, C = 16, 224, 224, 128
N_CORES = 8
B_LOC = B // N_CORES
QT = 4  # quarters per row-pair: 224 row-pairs * 4 = 896 = 7 * 128 lanes


def _legalize_waits(nc, max_waits=1):
    """This walrus build's ISA structs accept a single sync-wait command per
    instruction, but Tile's wait emission (not transitively minimal) can leave
    2+ waits.  Two-step fix, semantics-preserving:
      1. prune a wait when it is provably dominated through a kept wait
         (some instruction on the kept wait's engine proc, at/before the kept
         wait value, itself directly waits on the dropped semaphore at >= the
         dropped value);
      2. hoist any remaining extras onto same-engine NoOp instructions
         inserted immediately before (sequencer program order preserves the
         blocking semantics)."""
    import bass_rust
    from concourse.tile_scheduler import PROC_NAME_TO_IDX

    f = nc.m.functions[0]
    insts = [i for b in f.blocks for i in b.instructions]

    def pidx(ant_name):
        return PROC_NAME_TO_IDX[ant_name.rsplit("_", 1)[0]]

    by_proc = {}
    for i in insts:
        p = getattr(i, "bass_scheduled_proc", None)
        t = getattr(i, "bass_scheduled_tick", None)
        if p is None or t is None:
            continue
        by_proc.setdefault(p, []).append((t, i))
    for v in by_proc.values():
        v.sort(key=lambda x: x[0])

    def direct_waits(j):
        si = j.sync_info
        out = {}
        for w in si.on_wait if si else []:
            k = pidx(w.ant_name)
            out[k] = max(out.get(k, -1), w.wait_value)
        return out

    engine_procs = {v for k, v in PROC_NAME_TO_IDX.items()
                    if not k.startswith(("DMAHW", "DMASW", "Collectives"))}

    nop_ctr = [0]
    for b in f.blocks:
        new_insts = []
        for i in b.instructions:
            si = i.sync_info
            if not si or len(si.on_wait) <= max_waits:
                new_insts.append(i)
                continue
            # dedupe per-sem (keep max value)
            best = {}
            for w in si.on_wait:
                k = (w.sync_type, w.id)
                if k not in best or w.wait_value > best[k].wait_value:
                    best[k] = w
            kept = list(best.values())
            # drop same-proc self-waits: an engine instruction waiting on its
            # own proc's semaphore for a tick strictly below its own scheduled
            # tick is guaranteed by program order (the engine runs serially);
            # keeping it only stalls on the ~1us deferred sem-write of the
            # predecessor.
            own_p = getattr(i, "bass_scheduled_proc", None)
            own_t = getattr(i, "bass_scheduled_tick", None)
            if own_p is not None and own_t is not None and i.opcode != "DMACopy":
                kept = [w for w in kept
                        if not (pidx(w.ant_name) == own_p
                                and w.wait_value < own_t)]
            # step 1: transitive pruning
            for wd in list(kept):
                if len(kept) <= max_waits:
                    break
                wd_p, wd_v = pidx(wd.ant_name), wd.wait_value
                ok = False
                for via in kept:
                    if via is wd:
                        continue
                    via_p, via_v = pidx(via.ant_name), via.wait_value
                    if via_p not in engine_procs:
                        continue
                    for t, j in by_proc.get(via_p, []):
                        if t > via_v:
                            break
                        if direct_waits(j).get(wd_p, -1) >= wd_v:
                            ok = True
                            break
                    if ok:
                        break
                if ok:
                    kept.remove(wd)
            # step 2: hoist extras onto preceding same-engine NoOps
            while len(kept) > max_waits:
                w = kept.pop(0)
                nop = mybir.InstNoOp(name=f"I-waitnop-{nop_ctr[0]}", ins=[], outs=[])
                nop_ctr[0] += 1
                nop.engine = i.engine
                nop.sync_info = bass_rust.SyncInfo(on_wait=[w], on_update=[])
                new_insts.append(nop)
            si.on_wait = kept
            new_insts.append(i)
        b.instructions = new_insts
    return nc


def build_kernel(b_loc=B_LOC, h=H, w=W, c=C, qt=QT, legalize=True):
    ho = h // 2
    rowlen = w * c            # elems per input row
    qrow = rowlen // qt       # input elems per parity per lane-unit
    hp = b_loc * ho           # row-pairs in this shard
    hp_pb = 32 if hp % 32 == 0 else hp   # row-pairs per partition block
    assert hp % hp_pb == 0
    pn = hp_pb * qt           # partitions per block
    assert pn <= 128
    n_blocks = hp // hp_pb
    qc = 512 // c             # window-q units per PSUM group (g multiple of 512)

    # Load-chunks are big (2 MB/DMA keeps the SDMA engines at line rate) and
    # split into compute sub-chunks whose g = fl/2 is <= 1024 and a multiple
    # of 512 (PSUM bank width), so both PSUM sums (2 banks each)
    # double-buffer within the 8 banks; first block starts small for
    # pipeline fill.
    def subsplit(fl):
        subs = []
        while fl:
            s = min(2048, fl)
            subs.append(s)
            fl -= s
        return subs

    if qrow == 7168:
        first, rest = [1024, 2048, 4096], [4096, 3072]
    else:
        assert qrow <= 2048
        first = rest = [qrow]
    fl_max = max(max(first), max(rest))
    gmax = min(1024, fl_max // 2)

    nc = bass.Bass()
    x = nc.declare_dram_parameter("x", [b_loc, h, w, c], F32, isOutput=False)
    y = nc.declare_dram_parameter("y", [b_loc, ho, w // 2, c], F32, isOutput=True)

    # x viewed as [par(2), hp, qt, qrow]: batch rows are contiguous so (b h)
    # flattens seamlessly; partition p = (hp_local, qt).  par is outermost so
    # each chunk loads with two 3-dim DMAs (DMA APs are capped at 3 dims).
    xq = (
        x[:]
        .rearrange("b h w c -> (b h) (w c)")
        .rearrange("(hp par) f -> hp par f", par=2)
        .rearrange("hp par (qt s) -> par hp qt s", qt=qt)
    )
    # y viewed as [hp, qt, qrow/2]
    yq = (
        y[:]
        .rearrange("b h w c -> (b h) (w c)")
        .rearrange("hp (qt s) -> hp qt s", qt=qt)
    )

    mul = mybir.AluOpType.mult

    chunks = []
    for bi in range(n_blocks):
        off = 0
        for fl in (first if bi == 0 else rest):
            chunks.append((bi, off, fl))
            off += fl

    with ExitStack() as ctx:
        tc = ctx.enter_context(tile.TileContext(nc))
        con = ctx.enter_context(tc.tile_pool(name="con", bufs=1))
        iop = ctx.enter_context(tc.tile_pool(name="io", bufs=3))
        epp = ctx.enter_context(tc.tile_pool(name="ex", bufs=2))
        dfr = ctx.enter_context(tc.tile_pool(name="dfr", bufs=2))
        lnp = ctx.enter_context(tc.tile_pool(name="lnp", bufs=1))
        psp = ctx.enter_context(tc.psum_pool(name="ps", bufs=2))

        ident = con.tile([pn, pn], BF16, name="ident", tag="ident")
        make_identity(nc, ident[:])

        def load(k):
            bi, off, fl = chunks[k]
            hp0 = bi * hp_pb
            xin = iop.tile([pn, 2 * fl_max], F32, name="xin", tag="xin")
            for par in range(2):
                nc.sync.dma_start(
                    xin[:, par * fl:(par + 1) * fl],
                    xq[par, hp0:hp0 + hp_pb, :, off:off + fl],
                )
            return xin

        def pe_accum(dst, movings, g):
            """dst[:, 0:g] (PSUM f32) = sum of the 4 moving bf16 views, via
            identity-stationary matmuls accumulating per 512-wide bank group."""
            n_grp = (g + 511) // 512
            for j in range(n_grp):
                e0, e1 = 512 * j, min(512 * (j + 1), g)
                q0, q1 = qc * j, qc * j + (e1 - e0) // c
                for i, mv in enumerate(movings):
                    nc.tensor.matmul(
                        dst[:, e0:e1],
                        ident[:],
                        mv(q0, q1),
                        start=(i == 0),
                        stop=(i == len(movings) - 1),
                    )

        prev = None  # (s_ps, n_ps, dst, g) of the previous sub-chunk

        def emit_recip(st):
            lns = lnp.tile([pn, gmax], F32, name="lns", tag="lns")
            nc.scalar.activation(lns[:, 0:st[3]], st[0][:, 0:st[3]],
                                 mybir.ActivationFunctionType.Ln)
            r = dfr.tile([pn, gmax], BF16, name="r", tag="r")
            nc.scalar.activation(r[:, 0:st[3]], lns[:, 0:st[3]],
                                 mybir.ActivationFunctionType.Exp, scale=-1.0)
            return r

        def emit_tail(st, r):
            out_t = dfr.tile([pn, gmax], BF16, name="outt", tag="outt")
            nc.vector.tensor_tensor(out_t[:, 0:st[3]], st[1][:, 0:st[3]],
                                    r[:, 0:st[3]], mul)
            nc.gpsimd.dma_start(st[2], out_t[:, 0:st[3]])

        # prefetch two load-chunks deep: the issue of load k+2 only has to
        # clear chunk k-1's readers, so the transfer gets a full chunk period
        # to complete before exp k+2 needs it.
        xin = load(0)
        xin_next = load(1) if len(chunks) > 1 else None
        for k, (bi, off, fl) in enumerate(chunks):
            hp0 = bi * hp_pb

            xin_next2 = load(k + 2) if k + 2 < len(chunks) else None

            # ---- ACT: one exp over the whole load-chunk
            ex = epp.tile([pn, 2 * fl_max], BF16, name="ex", tag="ex")
            nc.scalar.activation(ex[:, 0:2 * fl], xin[:, 0:2 * fl],
                                 mybir.ActivationFunctionType.Exp)

            # ---- compute sub-chunks (even span [s0:s0+fs], odd [fl+s0:...])
            s0 = 0
            for fs in subsplit(fl):
                gl = fs // 2
                ql = fs // (2 * c)

                mAB = epp.tile([pn, 2048], BF16, name="mAB", tag="mAB")
                nc.vector.tensor_tensor(mAB[:, 0:fs], xin[:, s0:s0 + fs],
                                        ex[:, s0:s0 + fs], mul)

                # PE: s = EA+EB+EC+ED
                s_ps = psp.tile([pn, 1024], F32, name="s_ps", tag="s_ps")
                exv = ex[:, 0:2 * fl].rearrange(
                    "p (par q two c) -> p par q two c",
                    par=2, q=fl // (2 * c), two=2, c=c)
                qb = s0 // (2 * c)
                pe_accum(
                    s_ps,
                    [lambda q0, q1, par=par, tw=tw:
                     exv[:, par, qb + q0:qb + q1, tw, :]
                     for par in range(2) for tw in range(2)],
                    gl,
                )

                mCD = epp.tile([pn, 2048], BF16, name="mCD", tag="mCD")
                nc.vector.tensor_tensor(mCD[:, 0:fs],
                                        xin[:, fl + s0:fl + s0 + fs],
                                        ex[:, fl + s0:fl + s0 + fs], mul)

                # DVE tail of the previous sub-chunk: out = n * r, cast-store
                if prev is not None:
                    emit_tail(prev, prev_r)

                # PE: n = m1+m2+m3+m4
                n_ps = psp.tile([pn, 1024], F32, name="n_ps", tag="n_ps")
                mabv = mAB[:, 0:fs].rearrange("p (q two c) -> p q two c",
                                              q=ql, two=2, c=c)
                mcdv = mCD[:, 0:fs].rearrange("p (q two c) -> p q two c",
                                              q=ql, two=2, c=c)
                pe_accum(
                    n_ps,
                    [lambda q0, q1, v=v, tw=tw: v[:, q0:q1, tw, :]
                     for v in (mabv, mcdv) for tw in range(2)],
                    gl,
                )

                prev = (s_ps, n_ps,
                        yq[hp0:hp0 + hp_pb, :,
                           (off + s0) // 2:(off + s0) // 2 + gl], gl)
                # this sub's reciprocal goes on ACT *now*, ahead of the next
                # load-chunk's big exp, so the tail never queues behind it
                prev_r = emit_recip(prev)
                s0 += fs
            xin, xin_next = xin_next, xin_next2

        # drain: last sub-chunk's tail (its recip was emitted in-loop)
        emit_tail(prev, prev_r)

    return _legalize_waits(nc) if legalize else nc


def kernel(**inputs) -> np.ndarray:
    from concourse.bass_utils import run_bass_kernel_spmd

    x = inputs["x"]
    assert x.shape == (B, H, W, C) and x.dtype == np.float32
    nc = build_kernel()
    shards = x.reshape(N_CORES, B_LOC, H, W, C)
    in_maps = [{"x": np.ascontiguousarray(shards[i])} for i in range(N_CORES)]
    res = run_bass_kernel_spmd(nc, in_maps, list(range(N_CORES)))
    return np.concatenate([r["y"] for r in res.results], axis=0)


if __name__ == "__main__":
    # Small-shape CoreSim validation (no hardware).
    from concourse.bass_interp import CoreSim

    b_loc, h, w, c = 1, 8, 16, 128
    nc = build_kernel(b_loc, h, w, c, legalize=False)
    rng = np.random.default_rng(0)
    xs = rng.standard_normal((b_loc, h, w, c), dtype=np.float32)

    sim = CoreSim(nc)
    sim.tensor("x")[:] = xs
    sim.simulate()
    got = sim.tensor("y").copy()

    xd = xs.astype(np.float64)
    p = xd.reshape(b_loc, h // 2, 2, w // 2, 2, c).transpose(0, 1, 3, 2, 4, 5)
    p = p.reshape(b_loc, h // 2, w // 2, 4, c)
    e = np.exp(p - p.max(axis=3, keepdims=True))
    ref = (p * e).sum(axis=3) / e.sum(axis=3)
    err = np.abs(got - ref).max() / np.abs(ref).max()
    print("scale-rel err:", err, "max abs err:", np.abs(got - ref).max())
    assert err < 2e-2, "sim mismatch"
    print("SIM OK")


# revision 29
# speedup vs baseline: 1.0760x; 1.0760x over previous
"""AttMaxPool2D (2x2 softmax-attention pooling) Trainium2 Bass kernel.

Problem: x [16, 224, 224, 128] f32 NHWC -> out [16, 112, 112, 128]
  patches = 2x2 non-overlapping windows; out = sum(p * softmax(p, axis=window)).

Sharding: pure data parallel over batch: 8 cores x 2 examples each.

Layout: each SBUF partition owns a QUARTER of one output-row-pair
(224 row-pairs x 4 quarters = 896 units = 7 full blocks of 128 partitions, no
idle lanes).  Free dim = segments of the input row-pair quarter; even/odd
input row segments are packed [0:fl] / [fl:2fl] per partition.

The kernel is memory-bound (64.2 MB/core at ~360 GB/s ~= 178 us), so the
engine split keeps every compute engine under that roofline (tolerance gate
is 2e-2, so bf16 intermediates are fine):
  ACT: E = exp(x) (f32 -> bf16), then r = exp(-ln(s)) one chunk deferred
       (Ln/Exp share one table set; DVE iterative divide would be slower).
  DVE: only the products: mAB = x_even * E_even, mCD = x_odd * E_odd
       (f32 x bf16 -> bf16, 1x mode), and the final out = n * r.
  PE:  all window sums via identity-stationary matmuls accumulating into
       PSUM (fp32 accumulate, exact):  n = m1+m2+m3+m4, s = EA+EB+EC+ED.
       Group width 512 f32 = one PSUM bank; n and s each use 4 banks.
  Store: SWDGE (gpsimd) DMA with bf16->f32 cast; loads issued from SP (sync)
       so no DMA-issue time lands on ACT/DVE.
"""

import os
from contextlib import ExitStack

import numpy as np

import concourse.bass as bass
import concourse.mybir as mybir
import concourse.tile as tile
from concourse.masks import make_identity

F32 = mybir.dt.float32
BF16 = mybir.dt.bfloat16

# Full problem shape (hardcoded per contract).
B, H, W, C = 16, 224, 224, 128
N_CORES = 8
B_LOC = B // N_CORES
QT = 4  # quarters per row-pair: 224 row-pairs * 4 = 896 = 7 * 128 lanes


def _legalize_waits(nc, max_waits=1):
    """This walrus build's ISA structs accept a single sync-wait command per
    instruction, but Tile's wait emission (not transitively minimal) can leave
    2+ waits.  Two-step fix, semantics-preserving:
      1. prune a wait when it is provably dominated through a kept wait
         (some instruction on the kept wait's engine proc, at/before the kept
         wait value, itself directly waits on the dropped semaphore at >= the
         dropped value);
      2. hoist any remaining extras onto same-engine NoOp instructions
         inserted immediately before (sequencer program order preserves the
         blocking semantics)."""
    import bass_rust
    from concourse.tile_scheduler import PROC_NAME_TO_IDX

    f = nc.m.functions[0]
    insts = [i for b in f.blocks for i in b.instructions]

    def pidx(ant_name):
        return PROC_NAME_TO_IDX[ant_name.rsplit("_", 1)[0]]

    by_proc = {}
    for i in insts:
        p = getattr(i, "bass_scheduled_proc", None)
        t = getattr(i, "bass_scheduled_tick", None)
        if p is None or t is None:
            continue
        by_proc.setdefault(p, []).append((t, i))
    for v in by_proc.values():
        v.sort(key=lambda x: x[0])

    def direct_waits(j):
        si = j.sync_info
        out = {}
        for w in si.on_wait if si else []:
            k = pidx(w.ant_name)
            out[k] = max(out.get(k, -1), w.wait_value)
        return out

    engine_procs = {v for k, v in PROC_NAME_TO_IDX.items()
                    if not k.startswith(("DMAHW", "DMASW", "Collectives"))}

    nop_ctr = [0]
    for b in f.blocks:
        new_insts = []
        for i in b.instructions:
            si = i.sync_info
            if not si or len(si.on_wait) <= max_waits:
                new_insts.append(i)
                continue
            # dedupe per-sem (keep max value)
            best = {}
            for w in si.on_wait:
                k = (w.sync_type, w.id)
                if k not in best or w.wait_value > best[k].wait_value:
                    best[k] = w
            kept = list(best.values())
            # drop same-proc self-waits: an engine instruction waiting on its
            # own proc's semaphore for a tick strictly below its own scheduled
            # tick is guaranteed by program order (the engine runs serially);
            # keeping it only stalls on the ~1us deferred sem-write of the
            # predecessor.
            own_p = getattr(i, "bass_scheduled_proc", None)
            own_t = getattr(i, "bass_scheduled_tick", None)
            if own_p is not None and own_t is not None and i.opcode != "DMACopy":
                kept = [w for w in kept
                        if not (pidx(w.ant_name) == own_p
                                and w.wait_value < own_t)]
            # step 1: transitive pruning
            for wd in list(kept):
                if len(kept) <= max_waits:
                    break
                wd_p, wd_v = pidx(wd.ant_name), wd.wait_value
                ok = False
                for via in kept:
                    if via is wd:
                        continue
                    via_p, via_v = pidx(via.ant_name), via.wait_value
                    if via_p not in engine_procs:
                        continue
                    for t, j in by_proc.get(via_p, []):
                        if t > via_v:
                            break
                        if direct_waits(j).get(wd_p, -1) >= wd_v:
                            ok = True
                            break
                    if ok:
                        break
                if ok:
                    kept.remove(wd)
            # step 2: hoist extras onto preceding same-engine NoOps
            while len(kept) > max_waits:
                w = kept.pop(0)
                nop = mybir.InstNoOp(name=f"I-waitnop-{nop_ctr[0]}", ins=[], outs=[])
                nop_ctr[0] += 1
                nop.engine = i.engine
                nop.sync_info = bass_rust.SyncInfo(on_wait=[w], on_update=[])
                new_insts.append(nop)
            si.on_wait = kept
            new_insts.append(i)
        b.instructions = new_insts
    return nc


def build_kernel(b_loc=B_LOC, h=H, w=W, c=C, qt=QT, legalize=True):
    ho = h // 2
    rowlen = w * c            # elems per input row
    qrow = rowlen // qt       # input elems per parity per lane-unit
    hp = b_loc * ho           # row-pairs in this shard
    hp_pb = 32 if hp % 32 == 0 else hp   # row-pairs per partition block
    assert hp % hp_pb == 0
    pn = hp_pb * qt           # partitions per block
    assert pn <= 128
    n_blocks = hp // hp_pb
    qc = 512 // c             # window-q units per PSUM group (g multiple of 512)

    # Load-chunks are big (2 MB/DMA keeps the SDMA engines at line rate) and
    # split into compute sub-chunks whose g = fl/2 is <= 1024 and a multiple
    # of 512 (PSUM bank width), so both PSUM sums (2 banks each)
    # double-buffer within the 8 banks; first block starts small for
    # pipeline fill.
    def subsplit(fl):
        subs = []
        while fl:
            s = min(2048, fl)
            subs.append(s)
            fl -= s
        return subs

    if qrow == 7168:
        first, rest = [1024, 2048, 4096], [4096, 3072]
    else:
        assert qrow <= 2048
        first = rest = [qrow]
    fl_max = max(max(first), max(rest))
    gmax = min(1024, fl_max // 2)

    nc = bass.Bass()
    x = nc.declare_dram_parameter("x", [b_loc, h, w, c], F32, isOutput=False)
    y = nc.declare_dram_parameter("y", [b_loc, ho, w // 2, c], F32, isOutput=True)

    # x viewed as [par(2), hp, qt, qrow]: batch rows are contiguous so (b h)
    # flattens seamlessly; partition p = (hp_local, qt).  par is outermost so
    # each chunk loads with two 3-dim DMAs (DMA APs are capped at 3 dims).
    xq = (
        x[:]
        .rearrange("b h w c -> (b h) (w c)")
        .rearrange("(hp par) f -> hp par f", par=2)
        .rearrange("hp par (qt s) -> par hp qt s", qt=qt)
    )
    # y viewed as [hp, qt, qrow/2]
    yq = (
        y[:]
        .rearrange("b h w c -> (b h) (w c)")
        .rearrange("hp (qt s) -> hp qt s", qt=qt)
    )

    mul = mybir.AluOpType.mult

    chunks = []
    for bi in range(n_blocks):
        off = 0
        for fl in (first if bi == 0 else rest):
            chunks.append((bi, off, fl))
            off += fl

    with ExitStack() as ctx:
        tc = ctx.enter_context(tile.TileContext(nc))
        con = ctx.enter_context(tc.tile_pool(name="con", bufs=1))
        iop = ctx.enter_context(tc.tile_pool(name="io", bufs=3))
        epp = ctx.enter_context(tc.tile_pool(name="ex", bufs=2))
        dfr = ctx.enter_context(tc.tile_pool(name="dfr", bufs=2))
        lnp = ctx.enter_context(tc.tile_pool(name="lnp", bufs=1))
        psp = ctx.enter_context(tc.psum_pool(name="ps", bufs=2))

        ident = con.tile([pn, pn], BF16, name="ident", tag="ident")
        make_identity(nc, ident[:])

        def load(k):
            bi, off, fl = chunks[k]
            hp0 = bi * hp_pb
            xin = iop.tile([pn, 2 * fl_max], F32, name="xin", tag="xin")
            for par in range(2):
                nc.sync.dma_start(
                    xin[:, par * fl:(par + 1) * fl],
                    xq[par, hp0:hp0 + hp_pb, :, off:off + fl],
                )
            return xin

        def pe_accum(dst, movings, g):
            """dst[:, 0:g] (PSUM f32) = sum of the 4 moving bf16 views, via
            identity-stationary matmuls accumulating per 512-wide bank group."""
            n_grp = (g + 511) // 512
            for j in range(n_grp):
                e0, e1 = 512 * j, min(512 * (j + 1), g)
                q0, q1 = qc * j, qc * j + (e1 - e0) // c
                for i, mv in enumerate(movings):
                    nc.tensor.matmul(
                        dst[:, e0:e1],
                        ident[:],
                        mv(q0, q1),
                        start=(i == 0),
                        stop=(i == len(movings) - 1),
                    )

        prev = None  # (s_ps, n_ps, dst, g) of the previous sub-chunk

        def emit_recip(st):
            lns = lnp.tile([pn, gmax], F32, name="lns", tag="lns")
            nc.scalar.activation(lns[:, 0:st[3]], st[0][:, 0:st[3]],
                                 mybir.ActivationFunctionType.Ln)
            r = dfr.tile([pn, gmax], BF16, name="r", tag="r")
            nc.scalar.activation(r[:, 0:st[3]], lns[:, 0:st[3]],
                                 mybir.ActivationFunctionType.Exp, scale=-1.0)
            return r

        def emit_tail(st, r):
            out_t = dfr.tile([pn, gmax], BF16, name="outt", tag="outt")
            nc.vector.tensor_tensor(out_t[:, 0:st[3]], st[1][:, 0:st[3]],
                                    r[:, 0:st[3]], mul)
            nc.gpsimd.dma_start(st[2], out_t[:, 0:st[3]])

        # prefetch two load-chunks deep: the issue of load k+2 only has to
        # clear chunk k-1's readers, so the transfer gets a full chunk period
        # to complete before exp k+2 needs it.
        xin = load(0)
        xin_next = load(1) if len(chunks) > 1 else None
        for k, (bi, off, fl) in enumerate(chunks):
            hp0 = bi * hp_pb

            xin_next2 = load(k + 2) if k + 2 < len(chunks) else None

            # ---- ACT: the pending recip goes AHEAD of the big exp so the
            # previous sub's tail never queues behind it, then one exp over
            # the whole load-chunk
            r_first = emit_recip(prev) if prev is not None else None
            ex = epp.tile([pn, 2 * fl_max], BF16, name="ex", tag="ex")
            nc.scalar.activation(ex[:, 0:2 * fl], xin[:, 0:2 * fl],
                                 mybir.ActivationFunctionType.Exp)

            # ---- compute sub-chunks (even span [s0:s0+fs], odd [fl+s0:...])
            s0 = 0
            for si, fs in enumerate(subsplit(fl)):
                gl = fs // 2
                ql = fs // (2 * c)

                r = r_first if si == 0 else (
                    emit_recip(prev) if prev is not None else None)

                mAB = epp.tile([pn, 2048], BF16, name="mAB", tag="mAB")
                nc.vector.tensor_tensor(mAB[:, 0:fs], xin[:, s0:s0 + fs],
                                        ex[:, s0:s0 + fs], mul)

                # PE: s = EA+EB+EC+ED
                s_ps = psp.tile([pn, 1024], F32, name="s_ps", tag="s_ps")
                exv = ex[:, 0:2 * fl].rearrange(
                    "p (par q two c) -> p par q two c",
                    par=2, q=fl // (2 * c), two=2, c=c)
                qb = s0 // (2 * c)
                pe_accum(
                    s_ps,
                    [lambda q0, q1, par=par, tw=tw:
                     exv[:, par, qb + q0:qb + q1, tw, :]
                     for par in range(2) for tw in range(2)],
                    gl,
                )

                mCD = epp.tile([pn, 2048], BF16, name="mCD", tag="mCD")
                nc.vector.tensor_tensor(mCD[:, 0:fs],
                                        xin[:, fl + s0:fl + s0 + fs],
                                        ex[:, fl + s0:fl + s0 + fs], mul)

                # DVE tail of the previous sub-chunk: out = n * r, cast-store
                if prev is not None:
                    emit_tail(prev, r)

                # PE: n = m1+m2+m3+m4
                n_ps = psp.tile([pn, 1024], F32, name="n_ps", tag="n_ps")
                mabv = mAB[:, 0:fs].rearrange("p (q two c) -> p q two c",
                                              q=ql, two=2, c=c)
                mcdv = mCD[:, 0:fs].rearrange("p (q two c) -> p q two c",
                                              q=ql, two=2, c=c)
                pe_accum(
                    n_ps,
                    [lambda q0, q1, v=v, tw=tw: v[:, q0:q1, tw, :]
                     for v in (mabv, mcdv) for tw in range(2)],
                    gl,
                )

                prev = (s_ps, n_ps,
                        yq[hp0:hp0 + hp_pb, :,
                           (off + s0) // 2:(off + s0) // 2 + gl], gl)
                s0 += fs
            xin, xin_next = xin_next, xin_next2

        # drain: last sub-chunk's recip + tail
        r = emit_recip(prev)
        emit_tail(prev, r)

    return _legalize_waits(nc) if legalize else nc


def kernel(**inputs) -> np.ndarray:
    from concourse.bass_utils import run_bass_kernel_spmd

    x = inputs["x"]
    assert x.shape == (B, H, W, C) and x.dtype == np.float32
    nc = build_kernel()
    shards = x.reshape(N_CORES, B_LOC, H, W, C)
    in_maps = [{"x": np.ascontiguousarray(shards[i])} for i in range(N_CORES)]
    res = run_bass_kernel_spmd(nc, in_maps, list(range(N_CORES)))
    return np.concatenate([r["y"] for r in res.results], axis=0)


if __name__ == "__main__":
    # Small-shape CoreSim validation (no hardware).
    from concourse.bass_interp import CoreSim

    b_loc, h, w, c = 1, 8, 16, 128
    nc = build_kernel(b_loc, h, w, c, legalize=False)
    rng = np.random.default_rng(0)
    xs = rng.standard_normal((b_loc, h, w, c), dtype=np.float32)

    sim = CoreSim(nc)
    sim.tensor("x")[:] = xs
    sim.simulate()
    got = sim.tensor("y").copy()

    xd = xs.astype(np.float64)
    p = xd.reshape(b_loc, h // 2, 2, w // 2, 2, c).transpose(0, 1, 3, 2, 4, 5)
    p = p.reshape(b_loc, h // 2, w // 2, 4, c)
    e = np.exp(p - p.max(axis=3, keepdims=True))
    ref = (p * e).sum(axis=3) / e.sum(axis=3)
    err = np.abs(got - ref).max() / np.abs(ref).max()
    print("scale-rel err:", err, "max abs err:", np.abs(got - ref).max())
    assert err < 2e-2, "sim mismatch"
    print("SIM OK")
